# revision 11
# baseline (speedup 1.0000x reference)
"""Trainium2 Bass kernel for vertices_to_edges (gnn_message_passing).

out[b, c, e] = 0.5 * (VT[b, edges[b,e,0], c] + VT[b, edges[b,e,1], c])

Sharding: B=4 batches x 2 edge-halves -> 8 cores (data parallel).

Host pre-scales the table by 0.5, stores bf16 with 128 padded channels
(256B rows). Non-transpose GPSIMD dma_gather pulls endpoint rows as
[128, segs, 128] bf16 in 1024-index calls round-robined over the 4 SWDGE
queues (measured optimum ~2.3 ns/idx: the per-call queue rotation keeps
multiple HBM reads in flight per SDMA engine; the gather is HBM-latency-
not bandwidth-bound, so this is the roofline). Queue q's ucode pair only
reads idx partitions [32q, 32q+32), so instead of replicating the wrap16
index stream into all 8 windows (57.5 KB/partition) each queue's calls
are packed tightly into its own window (14.4 KB/partition) — the freed
SBUF deepens the gather pipeline to 9 tiles. PE transposes-and-adds
segment pairs into PSUM via regular bf16 identity matmuls (start/stop
accumulation performs v1+v2), ACT/DVE copy PSUM->SBUF as bf16, and the
channels-first output is written as FOUR quarter-tile [128, 1024] HWDGE
bursts per tile: full-128-partition writes spread each burst over all 16
SDMA engines (62-row writes skew engines 0-7), and the short bursts
interleave gently with the random gather reads — together these remove
the HBM read/write-turnaround penalty the write stream otherwise
inflicts on the gathers (1.46 ms -> 1.07 ms measured). Host unshard
upcasts to f32, slices the 62 real channels, and folds the sort
permutation.
"""

import numpy as np
import ml_dtypes

B, V, E, C = 4, 150000, 450000, 62
CPAD = 128  # bf16 channels padded: 256B rows
P = 128
N_CORES = 8
EH = E // 2  # 225000 edges per core
CHUNK_SHIFT = 15
CHUNK = 1 << CHUNK_SHIFT  # 32768
NCH = (V + CHUNK - 1) // CHUNK  # 5
TILE_E = 4096
K = TILE_E // P  # 32 segments per tile
MAX_IDX = 1024
NQ = 4

_CACHE = {}


def _plan(run_pad):
    """run_pad: [NCH*NCH] shared padded run sizes (multiples of 128).
    Returns (runs, s_pad, g1_calls, g2_calls, n_tiles, c1, c2).
    Calls are (s0, s1, chunk, queue, col): queue by global call ordinal,
    col = column offset into that queue's packed 32-partition idx window.
    c1/c2 = per-side packed column counts (shared across cores)."""
    runs = []
    s = 0
    for a in range(NCH):
        for b in range(NCH):
            n = int(run_pad[a * NCH + b])
            if n:
                runs.append([a, b, s, s + n])
                s += n
    s_pad = ((s + TILE_E - 1) // TILE_E) * TILE_E
    if s_pad > s:
        runs[-1][3] = s_pad
    n_tiles = s_pad // TILE_E

    spans = []
    for a, b, s0, s1 in runs:
        if spans and spans[-1][0] == a:
            spans[-1][2] = s1
        else:
            spans.append([a, s0, s1])

    def pieces(items, t):
        t0, t1 = t * TILE_E, (t + 1) * TILE_E
        out = []
        for base_chunk, s0, s1 in items:
            lo, hi = max(s0, t0), min(s1, t1)
            while lo < hi:
                mid = min(((lo - t0) // MAX_IDX + 1) * MAX_IDX + t0, hi)
                out.append((lo, mid, base_chunk))
                lo = mid
        return out

    g2_items = [(b, s0, s1) for a, b, s0, s1 in runs]
    ordinal = [0]
    qcols1 = [0] * NQ
    qcols2 = [0] * NQ

    def assign(raw, qcols):
        out = []
        for s0, s1, chunk in raw:
            q = ordinal[0] % NQ
            ordinal[0] += 1
            col = qcols[q]
            qcols[q] += (s1 - s0) // 16
            out.append((s0, s1, chunk, q, col))
        return out

    g1_calls, g2_calls = [], []
    for t in range(n_tiles):
        g1_calls.append(assign(pieces(spans, t), qcols1))
        g2_calls.append(assign(pieces(g2_items, t), qcols2))
    c1 = max(qcols1)
    c2 = max(qcols2)
    return runs, s_pad, g1_calls, g2_calls, n_tiles, c1, c2


def _build_module(s_pad, g1_calls, g2_calls, n_tiles, c1, c2, reps=1):
    import concourse.tile as tile
    from concourse import bacc, mybir

    nc = bacc.Bacc("TRN2", target_bir_lowering=False, debug=False, num_devices=N_CORES, num_swdge_queues=4)
    table = nc.dram_tensor("table", [V, CPAD], mybir.dt.bfloat16, kind="ExternalInput")
    i1 = nc.dram_tensor("i1", [128, c1], mybir.dt.int16, kind="ExternalInput")
    i2 = nc.dram_tensor("i2", [128, c2], mybir.dt.int16, kind="ExternalInput")
    idt = nc.dram_tensor("idt", [P, P], mybir.dt.bfloat16, kind="ExternalInput")
    out = nc.dram_tensor("out", [P, s_pad], mybir.dt.bfloat16, kind="ExternalOutput")

    def rows_of(chunk):
        return min(CHUNK, V - chunk * CHUNK)

    with tile.TileContext(nc) as tc:
        with (
            tc.tile_pool(name="idx", bufs=1) as idxp,
            tc.tile_pool(name="gat", bufs=9) as gatp,
            tc.tile_pool(name="psum", bufs=8, space="PSUM") as psump,
            tc.tile_pool(name="outp", bufs=4) as outp,
            tc.tile_pool(name="const", bufs=1) as constp,
        ):
            ident = constp.tile([P, P], mybir.dt.bfloat16)
            nc.sync.dma_start(ident[:], idt.ap())

            i1_sb = idxp.tile([128, c1], mybir.dt.int16)
            i2_sb = idxp.tile([128, c2], mybir.dt.int16)
            nc.sync.dma_start(i1_sb[:], i1.ap())
            nc.sync.dma_start(i2_sb[:], i2.ap())

            for _rep in range(reps):
                for t in range(n_tiles):
                    g1 = gatp.tile([P, K, CPAD], mybir.dt.bfloat16, tag="g1")
                    g2 = gatp.tile([P, K, CPAD], mybir.dt.bfloat16, tag="g2")
                    for g, calls, isb in ((g1, g1_calls[t], i1_sb), (g2, g2_calls[t], i2_sb)):
                        for s0, s1, chunk, q, col in calls:
                            seg0 = (s0 - t * TILE_E) // P
                            seg1 = (s1 - t * TILE_E + P - 1) // P
                            n = s1 - s0
                            nc.gpsimd.dma_gather(
                                out_ap=g[:, seg0:seg1, :],
                                in_ap=table.ap()[chunk * CHUNK : chunk * CHUNK + rows_of(chunk), :],
                                idxs_ap=isb[:, col : col + n // 16],
                                num_idxs=n,
                                num_idxs_reg=n,
                                elem_size=CPAD,
                                single_packet=False,
                                queue_num=q,
                            )
                    # full 128 partitions: the write burst spreads over all 16
                    # SDMA engines instead of skewing engines 0-7 (rows 62..127
                    # carry the zero pad channels).
                    o = outp.tile([P, TILE_E], mybir.dt.bfloat16, tag="o")
                    for qq in range(8):
                        ps = psump.tile([P, 512], mybir.dt.float32, space="PSUM", tag="ps")
                        for j4 in range(4):
                            j = qq * 4 + j4
                            # regular matmul vs identity: out[c,e] = sum_p g[p,c]*I[p,e]
                            # = g[e,c] — transposes AND start/stop-accumulates g1+g2.
                            nc.tensor.matmul(
                                out=ps[:, j4 * P : (j4 + 1) * P],
                                lhsT=g1[:, j, :],
                                rhs=ident[:],
                                start=True,
                                stop=False,
                            )
                            nc.tensor.matmul(
                                out=ps[:, j4 * P : (j4 + 1) * P],
                                lhsT=g2[:, j, :],
                                rhs=ident[:],
                                start=False,
                                stop=True,
                            )
                        osl = o[:, qq * 512 : (qq + 1) * 512]
                        if qq % 2 == 0:
                            nc.scalar.copy(osl, ps[:, :])
                        else:
                            nc.vector.tensor_copy(osl, ps[:, :])
                        # quarter-tile write bursts: short bursts interleave
                        # gently with the latency-bound random gather reads.
                        if qq % 2 == 1:
                            w0 = (qq - 1) * 512
                            nc.sync.dma_start(
                                out.ap()[:, t * TILE_E + w0 : t * TILE_E + w0 + 1024],
                                o[:, w0 : w0 + 1024],
                            )

    nc.compile()
    return nc


LAST_RESULT = None


def _pack_queue_windows(idx_flat, calls_by_tile, ncols):
    """Pack each call's wrap16 index block into its queue's 32-partition
    window (two 16-row tx/rx copies) at its assigned column offset."""
    host = np.zeros((128, ncols), np.int16)
    for calls in calls_by_tile:
        for s0, s1, chunk, q, col in calls:
            n = s1 - s0
            w = np.ascontiguousarray(idx_flat[s0:s1].reshape(-1, 16).T)  # [16, n/16]
            host[32 * q : 32 * q + 16, col : col + n // 16] = w
            host[32 * q + 16 : 32 * q + 32, col : col + n // 16] = w
    return host


def _prepare(inputs, reps=1):
    vertex_tokens = np.asarray(inputs["vertex_tokens"], dtype=np.float32)
    edges = np.asarray(inputs["edges"]).astype(np.int32)

    cores = []
    counts_all = np.zeros((N_CORES, NCH * NCH), dtype=np.int64)
    for core in range(N_CORES):
        b, half = divmod(core, 2)
        ed = edges[b, half * EH : (half + 1) * EH]
        v1, v2 = ed[:, 0], ed[:, 1]
        key = (v1 >> CHUNK_SHIFT) * NCH + (v2 >> CHUNK_SHIFT)
        order = np.argsort(key, kind="stable").astype(np.int32)
        counts_all[core] = np.bincount(key, minlength=NCH * NCH)
        cores.append((v1, v2, key, order))

    run_pad = ((counts_all.max(axis=0) + P - 1) // P) * P
    runs, s_pad, g1_calls, g2_calls, n_tiles, c1, c2 = _plan(run_pad)

    cache_key = (s_pad, str(g1_calls), str(g2_calls), reps)
    if cache_key not in _CACHE:
        _CACHE.clear()
        _CACHE[cache_key] = _build_module(s_pad, g1_calls, g2_calls, n_tiles, c1, c2, reps=reps)
    nc = _CACHE[cache_key]

    table_pad = np.zeros((B, V, CPAD), dtype=ml_dtypes.bfloat16)
    table_pad[:, :, :C] = (0.5 * vertex_tokens).astype(ml_dtypes.bfloat16)

    in_maps = []
    eslots = []
    for core in range(N_CORES):
        v1, v2, key, order = cores[core]
        counts = counts_all[core]
        idx1 = np.zeros(s_pad, dtype=np.int16)
        idx2 = np.zeros(s_pad, dtype=np.int16)
        eslot = np.full(s_pad, -1, dtype=np.int32)
        pos = 0
        for a, bb, s0, s1 in runs:
            n = int(counts[a * NCH + bb])
            seg = order[pos : pos + n]
            pos += n
            idx1[s0 : s0 + n] = (v1[seg] - (a << CHUNK_SHIFT)).astype(np.int16)
            idx2[s0 : s0 + n] = (v2[seg] - (bb << CHUNK_SHIFT)).astype(np.int16)
            eslot[s0 : s0 + n] = seg
        b, half = divmod(core, 2)
        in_maps.append(
            {
                "table": table_pad[b],
                "i1": _pack_queue_windows(idx1, g1_calls, c1),
                "i2": _pack_queue_windows(idx2, g2_calls, c2),
                "idt": np.eye(P, dtype=ml_dtypes.bfloat16),
            }
        )
        eslots.append(eslot)

    return nc, in_maps, eslots


def _unshard(results, eslots):
    out_ec = np.empty((B, E, C), dtype=np.float32)
    for core in range(N_CORES):
        b, half = divmod(core, 2)
        eslot = eslots[core]
        valid = eslot >= 0
        col_of_edge = np.empty(EH, dtype=np.int64)
        col_of_edge[eslot[valid]] = np.flatnonzero(valid)
        devT = results[core]["out"][:C].astype(np.float32).T  # [s_pad, 62]
        out_ec[b, half * EH : (half + 1) * EH, :] = devT[col_of_edge]
    return out_ec.transpose(0, 2, 1)


def kernel(**inputs) -> np.ndarray:
    global LAST_RESULT
    from concourse.bass_utils import run_bass_kernel_spmd

    nc, in_maps, eslots = _prepare(inputs)
    res = run_bass_kernel_spmd(nc, in_maps, core_ids=list(range(N_CORES)))
    LAST_RESULT = res
    return _unshard(res.results, eslots)
